# revision 23
# baseline (speedup 1.0000x reference)
"""BiSSM (bidirectional Mamba block) Trainium2 kernel, v6.

Contract: kernel(**inputs) takes the FULL unsharded inputs of
nn_BiSSMBlock (see setup_inputs) and returns the full (2, 1024, 1024)
output.  Sharding: (batch 2) x (direction 2) x (d_inner half 2) over 8
cores; a pairwise bf16 AllReduce between half-cores for the x_dbl
projection.  Host folds Wout into proj_W per direction and sums/flips
partials.

vs the v1 kernel: in_proj runs in bf16 (weights shipped as bf16 from
host), the per-state y accumulation runs on the PE array (identity /
diag(Dsk) matmuls accumulating in PSUM, replacing the DVE/Pool add
tree), dA for states 9/11/13/15 is squared from kept lower states
instead of re-exp'd, the f32 AllReduce overlaps the z-half of in_proj
(AllGather and bf16 collectives are ~1 ms/dispatch in this runtime —
avoid), scans stay on DVE (GpSimd scans and scalar ops are rejected by
the walrus codegen; GpSimd also cannot read PSUM), and the output
projection emits bf16 block partials summed on host.  Phase B
processes TWO states per instruction group (one broadcast DMA / dBu /
flat chain-broken scan / hc per pair) — real-HW per-instruction
overhead (~0.5 us, unmodeled by CoreSim) makes instruction count a
first-order cost.

Self-contained: only needs the concourse/bass toolchain at
/opt/trn_rl_repo and 8 visible neuron cores.
"""
import sys
sys.path.insert(0, "/opt/trn_rl_repo")
import numpy as np

import concourse.bass as bass
import concourse.bacc as bacc
import concourse.mybir as mybir
import concourse.tile as tile

F32 = mybir.dt.float32
F32R = mybir.dt.float32r
BF16 = mybir.dt.bfloat16
OP = mybir.AluOpType
AF = mybir.ActivationFunctionType

L = 1024          # sequence length
DM = 1024         # d_model
DH = 1024         # d_inner half per core
NG = 8            # channel groups of 128 (DH/128)
NST = 16          # d_state
TN = 512          # matmul moving-dim tile
NB = 4            # phase-B blocks
GB = 2            # groups per block

N_CORES = 8
CC_GROUPS = [[0, 1], [2, 3], [4, 5], [6, 7]]


def _build(structured_a, repeat=1, n_cores=N_CORES):
    nc = bacc.Bacc("TRN2", target_bir_lowering=False, debug=False, num_devices=n_cores)

    xT = nc.declare_dram_parameter("xT", [DM, L], BF16, isOutput=False)
    w_in = nc.declare_dram_parameter("w_in", [DM, 2 * DH], BF16, isOutput=False)
    wx = nc.declare_dram_parameter("wx", [DH, 96], BF16, isOutput=False)
    wdt = nc.declare_dram_parameter("wdt", [64, DH], BF16, isOutput=False)
    wco = nc.declare_dram_parameter("wco", [DH, DM], BF16, isOutput=False)
    convw = nc.declare_dram_parameter("convw", [128, NG, 4], F32, isOutput=False)
    dskdiag = nc.declare_dram_parameter("dskdiag", [128, NG, 128], BF16, isOutput=False)
    ident = nc.declare_dram_parameter("ident", [128, 128], BF16, isOutput=False)
    convb = nc.declare_dram_parameter("convb", [128, NG], F32, isOutput=False)
    binz = nc.declare_dram_parameter("binz", [128, NG], F32, isOutput=False)
    bdt = nc.declare_dram_parameter("bdt", [128, NG], F32, isOutput=False)
    A_ = nc.declare_dram_parameter("A_", [128, NG, NST], F32, isOutput=False)
    outp = nc.declare_dram_parameter("outp", [NB, DM, L], BF16, isOutput=True)

    ccs = [(nc.dram_tensor(f"cc_in{r}", [96, L], F32),
            nc.dram_tensor(f"cc_out{r}", [96, L], F32),
            nc.dram_tensor(f"cc_sum{r}", [32, L], BF16)) for r in range(repeat)]

    with tile.TileContext(nc) as tc:
        consts_cm = tc.tile_pool(name="consts", bufs=1)
        consts = consts_cm.__enter__()
        cw = consts.tile([128, NG, 4], F32)
        nc.sync.dma_start(out=cw[:], in_=convw[:])
        ddg = consts.tile([128, NG, 128], BF16)
        nc.sync.dma_start(out=ddg[:], in_=dskdiag[:])
        idt = consts.tile([128, 128], BF16)
        nc.sync.dma_start(out=idt[:], in_=ident[:])
        cb = consts.tile([128, NG], F32)
        nc.sync.dma_start(out=cb[:], in_=convb[:])
        bz = consts.tile([128, NG], F32)
        nc.sync.dma_start(out=bz[:], in_=binz[:])
        bd = consts.tile([128, NG], F32)
        nc.sync.dma_start(out=bd[:], in_=bdt[:])
        At = consts.tile([128, NG, NST], F32)
        nc.sync.dma_start(out=At[:], in_=A_[:])

        for rep in range(repeat):
            cc_in, cc_out, cc_sum = ccs[rep]
            _emit_one(nc, tc, structured_a, rep,
                      xT, w_in, wx, wdt, wco, outp, cc_in, cc_out, cc_sum,
                      cw, ddg, idt, cb, bz, bd, At)

        consts_cm.__exit__(None, None, None)

    nc.compile()
    return nc


def _emit_one(nc, tc, structured_a, rep,
              xT, w_in, wx, wdt, wco, outp, cc_in, cc_out, cc_sum,
              cw, ddg, idt, cb, bz, bd, At):
    w_in_r = w_in.ap().rearrange("(k p) m -> p k m", p=128)
    sfx = f"_{rep}"

    bigp_cm = tc.tile_pool(name="bigp" + sfx, bufs=1)
    bigp = bigp_cm.__enter__()
    u3 = bigp.tile([128, NG, L], BF16, tag="u3")
    szg = bigp.tile([128, NG, L], BF16, tag="szg")
    delta = bigp.tile([128, NG, L], F32, tag="delta")
    du = bigp.tile([128, NG, L], BF16, tag="du")

    # ---------------- Phase A: projections + conv ----------------
    poolA_cm = tc.tile_pool(name="poolA" + sfx, bufs=1)
    poolA = poolA_cm.__enter__()
    with tc.tile_pool(name="wchunk" + sfx, bufs=2) as wchunk, \
         tc.tile_pool(name="scrp" + sfx, bufs=2) as scrp, \
         tc.tile_pool(name="mm_ps" + sfx, bufs=2, space="PSUM") as mm_ps:
        xts = poolA.tile([128, 8, L], BF16, tag="xts")
        for k in range(8):
            nc.sync.dma_start(
                out=xts[:, k, :],
                in_=xT.ap().rearrange("(k p) t -> p k t", p=128)[:, k, :])
        xin = poolA.tile([128, NG, L + 3], BF16, tag="xin")
        halo = bass.AP(tensor=xin.tensor, offset=xin.offset,
                       ap=[[xin.ap[0][0], 128], [L + 3, NG], [1, 3]])
        nc.vector.memset(halo, 0.0)

        # xin half of in_proj, then depthwise conv via diagonal matmuls
        for m in range(8):
            wi = wchunk.tile([128, 8, 128], BF16, tag="wi")
            nc.sync.dma_start(out=wi[:], in_=w_in_r[:, :, m * 128:(m + 1) * 128])
            psx = mm_ps.tile([128, 2 * TN], F32, tag="psx", bufs=2)
            for tn in range(2):
                for k in range(8):
                    nc.tensor.matmul(psx[:, tn * TN:(tn + 1) * TN], wi[:, k, :],
                                     xts[:, k, tn * TN:(tn + 1) * TN],
                                     start=(k == 0), stop=(k == 7))
            nc.scalar.copy(out=xin[:, m, 3: 3 + L], in_=psx[:])
            cv_eng = nc.vector
            scr = scrp.tile([128, L], F32, tag="scr")
            cv_eng.tensor_scalar_mul(out=scr[:], in0=xin[:, m, 3:3 + L],
                                     scalar1=cw[:, m, 3:4])
            for k in range(3):
                cv_eng.scalar_tensor_tensor(
                    out=scr[:], in0=xin[:, m, k:k + L],
                    scalar=cw[:, m, k:k + 1], in1=scr[:],
                    op0=OP.mult, op1=OP.add)
            nc.scalar.activation(out=u3[:, m, :], in_=scr[:],
                                 func=AF.Silu, bias=cb[:, m:m + 1], scale=1.0)

        # x_dbl partial projection -> cc_in (bf16)
        wxs = poolA.tile([128, 8, 96], BF16, tag="wxs")
        nc.sync.dma_start(out=wxs[:], in_=wx.ap().rearrange("(k p) m -> p k m", p=128))
        xdb = poolA.tile([96, L], F32, tag="xdb")
        for tn in range(2):
            ps96 = mm_ps.tile([96, TN], F32, tag="ps96", bufs=2)
            for k in range(8):
                nc.tensor.matmul(ps96[:], wxs[:, k, :],
                                 u3[:, k, tn * TN:(tn + 1) * TN],
                                 start=(k == 0), stop=(k == 7))
            nc.scalar.copy(out=xdb[:, tn * TN:(tn + 1) * TN], in_=ps96[:])
        nc.sync.dma_start(out=cc_in[:], in_=xdb[:])

        # z half of in_proj: emitted before the collective so its PE/ACT
        # work fills the AllReduce wait window
        for m in range(8):
            wi = wchunk.tile([128, 8, 128], BF16, tag="wi")
            nc.sync.dma_start(out=wi[:], in_=w_in_r[:, :, DH + m * 128: DH + (m + 1) * 128])
            psz = mm_ps.tile([128, 2 * TN], F32, tag="psx", bufs=2)
            for tn in range(2):
                for k in range(8):
                    nc.tensor.matmul(psz[:, tn * TN:(tn + 1) * TN], wi[:, k, :],
                                     xts[:, k, tn * TN:(tn + 1) * TN],
                                     start=(k == 0), stop=(k == 7))
            nc.scalar.activation(out=szg[:, m, :], in_=psz[:], func=AF.Silu,
                                 bias=bz[:, m:m + 1], scale=1.0)

        nc.gpsimd.collective_compute("AllReduce", OP.add, replica_groups=CC_GROUPS,
                                     ins=[cc_in[:]], outs=[cc_out[:]])
        sum32 = poolA.tile([96, L], F32, tag="sum32")
        nc.sync.dma_start(out=sum32[0:64, :],
                          in_=bass.AP(tensor=cc_out, offset=0, ap=[[L, 64], [1, L]]))
        nc.scalar.dma_start(out=sum32[64:96, :],
                            in_=bass.AP(tensor=cc_out, offset=64 * L, ap=[[L, 32], [1, L]]))
        dt16 = poolA.tile([64, L], BF16, tag="dt16")
        nc.vector.tensor_copy(dt16[:], sum32[0:64, :])
        bc16 = poolA.tile([32, L], BF16, tag="bc16")
        nc.vector.tensor_copy(bc16[:], sum32[64:96, :])
        nc.sync.dma_start(out=cc_sum[:], in_=bc16[:])

        # delta = softplus(dt @ Wdt.T + bdt), via exp then ln(1+x)
        wds = poolA.tile([64, NG, 128], BF16, tag="wds")
        nc.sync.dma_start(out=wds[:], in_=wdt.ap().rearrange("k (g p) -> k g p", p=128))
        for g in range(NG):
            psd = mm_ps.tile([128, 2 * TN], F32, tag="psx", bufs=2)
            for tn in range(2):
                nc.tensor.matmul(psd[:, tn * TN:(tn + 1) * TN], wds[:, g, :],
                                 dt16[:, tn * TN:(tn + 1) * TN],
                                 start=True, stop=True)
            nc.scalar.activation(out=delta[:, g, :], in_=psd[:],
                                 func=AF.Exp, bias=bd[:, g:g + 1], scale=1.0)
            if g % 2 == 1:
                nc.scalar.activation(out=delta[:, g - 1:g + 1, :], in_=delta[:, g - 1:g + 1, :],
                                     func=AF.Ln, bias=1.0, scale=1.0)
                nc.vector.tensor_tensor(out=du[:, g - 1:g + 1, :], in0=delta[:, g - 1:g + 1, :],
                                        in1=u3[:, g - 1:g + 1, :], op=OP.mult)
    poolA_cm.__exit__(None, None, None)

    # ---------------- Phase B: selective scan + out_proj ----------------
    with tc.tile_pool(name="wcp" + sfx, bufs=1) as wcp, \
         tc.tile_pool(name="bcpool" + sfx, bufs=3) as bcpool, \
         tc.tile_pool(name="sA" + sfx, bufs=2) as sA, \
         tc.tile_pool(name="dAkp" + sfx, bufs=1) as dAkp, \
         tc.tile_pool(name="sB" + sfx, bufs=2) as sB, \
         tc.tile_pool(name="sH" + sfx, bufs=3) as sH, \
         tc.tile_pool(name="ygbp" + sfx, bufs=2) as ygbp, \
         tc.tile_pool(name="oslp" + sfx, bufs=4) as oslp, \
         tc.tile_pool(name="ps_y" + sfx, bufs=4, space="PSUM") as psy_pool, \
         tc.tile_pool(name="op_ps" + sfx, bufs=4, space="PSUM") as op_ps:
        wco_r = wco.ap().rearrange("(k p) m -> p k m", p=128)
        outp_r = outp.ap().rearrange("b (m p) t -> b p m t", p=128)
        wc = []
        for mc in range(2):
            w_ = wcp.tile([128, 8, TN], BF16, tag=f"wc{mc}")
            nc.scalar.dma_start(out=w_[:], in_=wco_r[:, :, mc * TN:(mc + 1) * TN])
            wc.append(w_)

        idx = 0
        for blk in range(NB):
            g0 = blk * GB
            dAkeep = {}
            psy = [psy_pool.tile([128, TN], F32, tag="psy", name=f"psy{rep}_{blk}_{i}")
                   for i in range(4)]
            # tslice ts covers (group g0 + ts//2, columns (ts%2)*TN)
            for ts in range(4):
                g = g0 + ts // 2
                t0 = (ts % 2) * TN
                nc.tensor.matmul(psy[ts][:], ddg[:, g, :], u3[:, g, t0:t0 + TN],
                                 start=True, stop=False)
            for np_ in range(NST // 2):
                n0 = 2 * np_  # states n0, n0+1 processed together
                bcp = bcpool.tile([128, 2, 2, L], BF16, tag="bcp")
                nc.sync.dma_start(
                    out=bcp[:],
                    in_=bass.AP(tensor=cc_sum, offset=2 * n0 * L,
                                ap=[[0, 128], [L, 4], [1, L]]))
                pstr = bcp.ap[0][0]
                # (state, group-rep, t) views of the B and C rows
                brep = bass.AP(tensor=bcp.tensor, offset=bcp.offset,
                               ap=[[pstr, 128], [2 * L, 2], [0, GB], [1, L]])
                crep = bass.AP(tensor=bcp.tensor, offset=bcp.offset + L,
                               ap=[[pstr, 128], [2 * L, 2], [0, GB], [1, L]])
                if structured_a and n0 in (4, 6):
                    dA = dAkp.tile([128, 2, GB, L], BF16, tag=f"dAk{n0}",
                                   name=f"dAk{rep}_{blk}_{n0}")
                    dAkeep[n0] = dA
                else:
                    dA = sA.tile([128, 2, GB, L], BF16, tag="dA")
                for s in range(2):
                    n = n0 + s
                    if structured_a and n in (9, 11, 13, 15):
                        kp = dAkeep[4 if n in (9, 11) else 6]
                        srcs = kp[:, (n - 9) // 2 % 2, :, :]
                        sq_eng = nc.vector if n in (9, 13) else nc.gpsimd
                        sq_eng.tensor_tensor(out=dA[:, s, :, :], in0=srcs, in1=srcs,
                                             op=OP.mult)
                    elif structured_a:
                        nc.scalar.activation(out=dA[:, s, :, :],
                                             in_=delta[:, g0:g0 + GB, :],
                                             func=AF.Exp, bias=0.0, scale=-float(n + 1))
                    else:
                        for gg in range(GB):
                            nc.scalar.activation(out=dA[:, s, gg, :],
                                                 in_=delta[:, g0 + gg, :],
                                                 func=AF.Exp, bias=0.0,
                                                 scale=At[:, g0 + gg, n:n + 1])
                dBu = sB.tile([128, 2, GB, L], BF16, tag="dBu")
                durep = bass.AP(tensor=du.tensor, offset=du.offset + g0 * L,
                                ap=[[du.ap[0][0], 128], [0, 2], [L, GB], [1, L]])
                nc.gpsimd.tensor_tensor(out=dBu[:], in0=durep, in1=brep, op=OP.mult)
                h = sH.tile([128, 2, GB, L], BF16, tag="h")
                # one flat scan over (2 states x GB groups); break the chain at
                # each interior segment start via dA[.., 0] = 0
                zpos = bass.AP(tensor=dA.tensor, offset=dA.offset + L,
                               ap=[[dA.ap[0][0], 128], [L, 2 * GB - 1], [1, 1]])
                nc.gpsimd.memset(zpos, 0.0)
                flat = lambda t: bass.AP(tensor=t.tensor, offset=t.offset,
                                         ap=[[t.ap[0][0], 128], [1, 2 * GB * L]])
                nc.vector.tensor_tensor_scan(flat(h), flat(dA), flat(dBu),
                                             0.0, OP.mult, OP.add)
                hc_eng = nc.gpsimd if (np_ % 3 == 2) else nc.vector
                hc_eng.tensor_tensor(out=h[:], in0=h[:], in1=crep, op=OP.mult)
                for s in range(2):
                    for ts in range(4):
                        nc.tensor.matmul(psy[ts][:], idt[:],
                                         h[:, s, ts // 2, (ts % 2) * TN:(ts % 2) * TN + TN],
                                         start=False, stop=(n0 + s == NST - 1))
                idx += 1
            # gate with silu(z) and project this block's channels
            ygb = ygbp.tile([128, GB, L], BF16, tag="ygb")
            for ts in range(4):
                g = g0 + ts // 2
                t0 = (ts % 2) * TN
                nc.vector.tensor_tensor(out=ygb[:, ts // 2, t0:t0 + TN], in0=psy[ts][:],
                                          in1=szg[:, g, t0:t0 + TN], op=OP.mult)
            for mc in range(2):
                for mm in range(4):
                    m = mc * 4 + mm
                    for tn in range(2):
                        ps = op_ps.tile([128, TN], F32, tag="ps_o")
                        for kk in range(GB):
                            nc.tensor.matmul(ps[:], wc[mc][:, g0 + kk, mm * 128:(mm + 1) * 128],
                                             ygb[:, kk, tn * TN:(tn + 1) * TN],
                                             start=(kk == 0), stop=(kk == GB - 1))
                        osl = oslp.tile([128, TN], BF16, tag="osl")
                        if (mm + tn) % 2 == 0:
                            nc.scalar.copy(out=osl[:], in_=ps[:])
                        else:
                            nc.vector.tensor_copy(osl[:], ps[:])
                        nc.sync.dma_start(out=outp_r[blk, :, m, tn * TN:(tn + 1) * TN],
                                          in_=osl[:])
    bigp_cm.__exit__(None, None, None)


def _prep_core_inputs(inputs, b, d, h):
    pref = "f_" if d == 0 else "b_"
    g = lambda k: np.asarray(inputs[pref + k], dtype=np.float32)
    x = np.asarray(inputs["x"], dtype=np.float32)[b]
    if d == 1:
        x = x[::-1]
    sl = slice(h * DH, (h + 1) * DH)

    Win = g("Win")
    w_in = np.concatenate([Win[sl].T, Win[2048 + h * DH: 2048 + (h + 1) * DH].T], axis=1)
    convw = g("convw")[sl]
    convb_eff = g("convb")[sl] + g("bin")[sl] * convw.sum(-1)
    pg = lambda v: np.ascontiguousarray(v.reshape(NG, 128).T)
    pg3 = lambda v: np.ascontiguousarray(v.reshape(NG, 128, -1).transpose(1, 0, 2))
    A = -np.exp(g("Alog")[sl])
    proj_W = np.asarray(inputs["proj_W"], dtype=np.float32)
    Pd = proj_W[:, d * DM:(d + 1) * DM]
    wco = (Pd @ g("Wout"))[:, sl].T

    # x_dbl rows permuted: dt 0:64 unchanged, then B/C interleaved
    Wx = g("Wx")[:, sl]
    perm = list(range(64)) + [64 + 16 * (i % 2) + i // 2 for i in range(32)]
    Wx = Wx[perm]

    # diagonal weight matrices for the D-skip matmuls
    dskdiag = np.zeros((128, NG, 128), np.float32)
    dk = pg(g("Dsk")[sl])    # [128, NG]
    rng = np.arange(128)
    for gi in range(NG):
        dskdiag[rng, gi, rng] = dk[:, gi]

    import ml_dtypes
    bf = lambda v: np.ascontiguousarray(v).astype(ml_dtypes.bfloat16)
    return {
        "xT": bf(x.T),
        "w_in": bf(w_in),
        "wx": bf(Wx.T),
        "wdt": bf(g("Wdt")[sl].T),
        "wco": bf(wco),
        "convw": pg3(convw),
        "dskdiag": bf(dskdiag),
        "ident": bf(np.eye(128, dtype=np.float32)),
        "convb": pg(convb_eff),
        "binz": pg(g("bin")[2048 + h * DH: 2048 + (h + 1) * DH]),
        "bdt": pg(g("bdt")[sl]),
        "A_": pg3(A),
    }


def _check_structured_a(inputs):
    ar = np.log(np.arange(1, NST + 1, dtype=np.float32))
    for pref in ("f_", "b_"):
        Alog = np.asarray(inputs[pref + "Alog"], dtype=np.float32)
        if not np.allclose(Alog, np.broadcast_to(ar, Alog.shape), atol=1e-5):
            return False
    return True


_CACHE = {}


def _get_nc(structured_a, repeat=1):
    key = ("v2", structured_a, repeat)
    if key not in _CACHE:
        _CACHE[key] = _build(structured_a, repeat=repeat)
    return _CACHE[key]


def kernel(**inputs):
    from concourse.bass_utils import run_bass_kernel_spmd

    nc = _get_nc(_check_structured_a(inputs))
    in_maps = []
    for c in range(N_CORES):
        b, d, h = c >> 2, (c >> 1) & 1, c & 1
        in_maps.append(_prep_core_inputs(inputs, b, d, h))
    res = run_bass_kernel_spmd(nc, in_maps, list(range(N_CORES)))
    partials = [np.asarray(res.results[c]["outp"], dtype=np.float32).sum(axis=0)
                for c in range(N_CORES)]

    B = 2
    out = np.zeros((B, L, DM), np.float32)
    for b in range(B):
        for d in range(2):
            s = (partials[b * 4 + d * 2 + 0] + partials[b * 4 + d * 2 + 1]).T
            if d == 1:
                s = s[::-1]
            out[b] += s
    proj_W = np.asarray(inputs["proj_W"], dtype=np.float32)
    bias = (np.asarray(inputs["f_bout"], dtype=np.float32) @ proj_W[:, :DM].T
            + np.asarray(inputs["b_bout"], dtype=np.float32) @ proj_W[:, DM:].T
            + np.asarray(inputs["proj_b"], dtype=np.float32))
    return out + bias
